# revision 3
# baseline (speedup 1.0000x reference)
"""Binary tree-LSTM (BinaryTokenTreeModel) Trainium2 kernel.

Problem: complete binary tree, depth 15 (N=32767 nodes), tree-LSTM with
state size 2H=512, gates 4*2H=2048, vocab 32.  Reference processes nodes
leaves-first; node i's input state is the concat of the first H=256 dims
of its two children's states.

Strategy (8 NeuronCores):
  * Data-parallel over 8 subtrees rooted at the 8 level-3 nodes (7..14).
    Each core runs a level-synchronous scan over its subtree (levels
    13..3 of the global tree), 2047 non-leaf nodes per core.
  * VOCAB=32 => x_proj = (W_ih @ emb.T + b) gathered by type: a 32-column
    table, folded into the level matmul as a one-hot contraction block
    (K = 256+256+32 = 544).
  * Leaf states take only 32 distinct values: precomputed tables
    (host, O(32) work).  Level 13's whole input contraction collapses to
    K=96 of one-hots, and the 16384 leaf output rows are a host-side
    gather of the 32-row table (zero arithmetic).
  * Top 7 nodes (levels 2..0): each core emits its subtree-root (h, c);
    the host finishes the 7-node chain in numpy (15 MFLOP, exact fp32).
    Collectives are avoided entirely (NRT profiling hangs on CC NEFFs).
  * Matmul operands are float16 (full-rate 1 col/cycle PE streaming,
    ~5e-4 rounding on W and h only); accumulation and all elementwise
    math stay fp32.

Self-contained: hardcodes all shapes; only needs numpy + the concourse
(bass) toolchain that ships with the environment.
"""

import sys

for _p in ("/opt/trn_rl_repo", "/root/.axon_site/_ro/trn_rl_repo"):
    if _p not in sys.path:
        sys.path.append(_p)

import numpy as np

import concourse.bacc as bacc
import concourse.mybir as mybir
import concourse.tile as tile
from concourse.bass_utils import run_bass_kernel_spmd

F32 = mybir.dt.float32
F16 = mybir.dt.float16
AF = mybir.ActivationFunctionType

N_CORES = 8
N = 32767
H = 256
H2 = 512
G = 2048  # 4 * H2
V = 32
LEAF0 = (1 << 14) - 1  # 16383: first leaf node id

# Gate column order: critical half (state dims 0:256) then deferred half
# (dims 256:512); within each half [i f o g] so sigmoid spans cols 0:768
# and tanh cols 768:1024 of each 1024-wide half.
GATE_PERM = np.concatenate([
    np.arange(0, 256), np.arange(512, 768),          # i_c f_c
    np.arange(1536, 1792), np.arange(1024, 1280),    # o_c g_c
    np.arange(256, 512), np.arange(768, 1024),       # i_d f_d
    np.arange(1792, 2048), np.arange(1280, 1536),    # o_d g_d
])

# (level, nodes-per-core, output row offset in the per-core out tensor)
PLAN = [
    (13, 1024, 0), (12, 512, 1024), (11, 256, 1536), (10, 128, 1792),
    (9, 64, 1920), (8, 32, 1984), (7, 16, 2016), (6, 8, 2032),
    (5, 4, 2040), (4, 2, 2044), (3, 1, 2046),
]
OUT_ROWS = 2048  # 2047 h rows + 1 root-c row
OHS_OFF = {12: 0, 11: 512, 10: 768, 9: 896, 8: 960, 7: 992, 6: 1008,
           5: 1016, 4: 1020, 3: 1022}

_BUILT = None  # cached (nc, input_names)
LAST_RESULTS = None  # BassKernelResults of the most recent run (for profiling)


def _sigmoid(x):
    return 1.0 / (1.0 + np.exp(-x))


class _Stor:
    """Per-level stationary-input storage (filled by the child level)."""

    def __init__(self, nc, L, M):
        self.M = M
        nch = max(1, (M + 127) // 128)
        mk = lambda n, sh, dt: nc.alloc_sbuf_tensor(f"{n}_{L}", sh, dt).ap()
        self.sA0 = mk("sA0", [128, M], F16)
        self.sA1 = mk("sA1", [128, M], F16)
        self.sB0 = mk("sB0", [128, M], F16)
        self.sB1 = mk("sB1", [128, M], F16)
        self.cin = mk("cin", [min(128, M), nch * 512], F32)


def _build_program(nc):
    din = {}
    for name, shape in [
        ("wk0", [128, G]), ("wk1", [128, G]), ("wk2", [128, G]), ("wk3", [128, G]),
        ("woh", [32, G]), ("w13", [96, G]),
        ("oh3", [96, 1024]), ("ohs", [32, 1023]),
    ]:
        din[name] = nc.dram_tensor(name, shape, F16, kind="ExternalInput").ap()
    din["eye"] = nc.dram_tensor("eye", [128, 128], F32, kind="ExternalInput").ap()
    din["cin13"] = nc.dram_tensor("cin13", [1024, 512], F32, kind="ExternalInput").ap()
    out_d = nc.dram_tensor("out", [OUT_ROWS, 512], F32, kind="ExternalOutput").ap()

    sb = lambda n, sh: nc.alloc_sbuf_tensor(n, sh, F32).ap()
    sbh = lambda n, sh: nc.alloc_sbuf_tensor(n, sh, F16).ap()
    wk = [sbh(f"wk{i}_s", [128, G]) for i in range(4)]
    woh_s = sbh("woh_s", [32, G])
    w13_s = sbh("w13_s", [96, G])
    oh3_s = sbh("oh3_s", [96, 1024])
    ohs_s = sbh("ohs_s", [32, 1023])
    eye_s = sb("eye_s", [128, 128])
    cin13_s = sb("cin13_s", [128, 8 * 512])

    stor = {L: _Stor(nc, L, M) for (L, M, _) in PLAN if L != 13}

    with tile.TileContext(nc) as tc:
        import contextlib

        with contextlib.ExitStack() as ctx:
            gc_pool = ctx.enter_context(
                tc.tile_pool(name="gc", bufs=2, space="PSUM"))
            gd_pool = ctx.enter_context(
                tc.tile_pool(name="gd", bufs=2, space="PSUM"))
            sig_pool = ctx.enter_context(tc.tile_pool(name="sig", bufs=3))
            cell_pool = ctx.enter_context(tc.tile_pool(name="cell", bufs=2))

            # weight / one-hot loads; L13's operands first, halves split
            # across the two HWDGE queues (sync + scalar)
            nc.sync.dma_start(w13_s[0:48], din["w13"][0:48])
            nc.scalar.dma_start(w13_s[48:96], din["w13"][48:96])
            nc.sync.dma_start(oh3_s[0:48], din["oh3"][0:48])
            nc.scalar.dma_start(oh3_s[48:96], din["oh3"][48:96])
            for k in range(8):
                (nc.scalar if k % 2 else nc.sync).dma_start(
                    cin13_s[:, k * 512:(k + 1) * 512],
                    din["cin13"][k * 128:(k + 1) * 128, :])
            for d, s in [
                (din["wk0"], wk[0]), (din["wk2"], wk[2]),
                (din["woh"], woh_s), (din["eye"], eye_s),
            ]:
                nc.sync.dma_start(s, d)
            for d, s in [
                (din["wk1"], wk[1]), (din["wk3"], wk[3]),
                (din["ohs"], ohs_s),
            ]:
                nc.scalar.dma_start(s, d)

            # HAM warm-up: ~12 junk matmuls as soon as w13 lands keep the
            # PE busy through the cold window so L13 runs at 2.4 GHz
            wtile = gc_pool.tile([128, 1024], F32, tag="gc")
            for _ in range(12):
                nc.tensor.matmul(wtile[0:128, 0:512], w13_s[:, 0:128],
                                 w13_s[:, 0:512], start=True, stop=True,
                                 skip_group_check=True)

            def feed_parent(parent, gtile, hsrc, csrc, P, ci):
                """Write child chunk crit states into parent stationary storage.

                Transposes reuse a dead gates PSUM tile of the same chunk
                (banks 0 and 1): the defer tile on fused levels (freed right
                after sig_d/tg_d), the crit tile on split levels.
                hsrc: [P, 256] h crit; csrc: [P, >=256] cols 0:256 c crit."""
                half = P // 2
                base = ci * 64
                t0 = gtile[0:128, 0:P]
                nc.tensor.transpose(t0, hsrc[:, 0:128], eye_s[0:P, 0:P])
                t1 = gtile[0:128, 512:512 + P]
                nc.tensor.transpose(t1, hsrc[:, 128:256], eye_s[0:P, 0:P])
                nc.vector.tensor_copy(parent.sA0[:, base:base + half], t0[:, 0:P:2])
                nc.vector.tensor_copy(parent.sA1[:, base:base + half], t1[:, 0:P:2])
                nc.vector.tensor_copy(parent.sB0[:, base:base + half], t0[:, 1:P:2])
                nc.vector.tensor_copy(parent.sB1[:, base:base + half], t1[:, 1:P:2])
                dr = base % 128
                cb = (ci // 2) * 512
                nc.sync.dma_start(parent.cin[dr:dr + half, cb:cb + 256],
                                  csrc[0:P:2, 0:256])
                nc.sync.dma_start(parent.cin[dr:dr + half, cb + 256:cb + 512],
                                  csrc[1:P:2, 0:256])

            def emit_mms(gtile, lhs_tiles, ws, col0, P):
                """k-outer accumulation of one 1024-col gate half."""
                nk = len(lhs_tiles)
                for k in range(nk):
                    for b in range(2):
                        nc.tensor.matmul(
                            gtile[0:P, b * 512:(b + 1) * 512],
                            lhs_tiles[k],
                            ws[k][:, col0 + b * 512:col0 + (b + 1) * 512],
                            start=(k == 0), stop=(k == nk - 1),
                            skip_group_check=True)

            for (L, M, row_off) in PLAN:
                nch = max(1, (M + 127) // 128)
                fused = M >= 256
                feeds = []
                for pk in range(nch):
                    P = min(128, M - pk * 128)
                    c0 = pk * 128
                    gc = gc_pool.tile([128, 1024], F32)
                    gd = gd_pool.tile([128, 1024], F32)
                    if L == 13:
                        lhs_tiles = [oh3_s[:, c0:c0 + P]]
                        ws = [w13_s]
                        cin_ap = cin13_s[0:P, pk * 512:(pk + 1) * 512]
                    else:
                        st = stor[L]
                        oh_ap = ohs_s[:, OHS_OFF[L] + c0:OHS_OFF[L] + c0 + P]
                        lhs_tiles = [st.sA0[:, c0:c0 + P], st.sA1[:, c0:c0 + P],
                                     st.sB0[:, c0:c0 + P], st.sB1[:, c0:c0 + P],
                                     oh_ap]
                        ws = wk + [woh_s]
                        cin_ap = st.cin[0:P, pk * 512:(pk + 1) * 512]
                    emit_mms(gc, lhs_tiles, ws, 0, P)
                    emit_mms(gd, lhs_tiles, ws, 1024, P)

                    cnew = cell_pool.tile([128, 512], F32)
                    hnew = cell_pool.tile([128, 512], F32)
                    cin3 = cin_ap.rearrange("p (j c) -> p j c", j=2)

                    if fused:
                        sig = sig_pool.tile([128, 1536], F32)
                        nc.scalar.activation(sig[0:P, 0:768], gc[0:P, 0:768],
                                             AF.Sigmoid)
                        nc.scalar.activation(sig[0:P, 768:1536], gd[0:P, 0:768],
                                             AF.Sigmoid)
                        tg = cell_pool.tile([128, 512], F32)
                        nc.scalar.activation(tg[0:P, 0:256], gc[0:P, 768:1024],
                                             AF.Tanh)
                        nc.scalar.activation(tg[0:P, 256:512], gd[0:P, 768:1024],
                                             AF.Tanh)
                        sig3 = sig[0:P].rearrange("p (j c) -> p j c", j=2)
                        tg3 = tg[0:P].rearrange("p (j c) -> p j c", j=2)
                        prod = cell_pool.tile([128, 512], F32)
                        prod3 = prod[0:P].rearrange("p (j c) -> p j c", j=2)
                        nc.vector.tensor_mul(prod3, sig3[:, :, 0:256], tg3)
                        fc = cell_pool.tile([128, 512], F32)
                        fc3 = fc[0:P].rearrange("p (j c) -> p j c", j=2)
                        nc.gpsimd.tensor_mul(fc3, sig3[:, :, 256:512], cin3)
                        nc.vector.tensor_add(cnew[0:P], fc[0:P], prod[0:P])
                        tcc = cell_pool.tile([128, 512], F32)
                        nc.scalar.activation(tcc[0:P], cnew[0:P], AF.Tanh)
                        tcc3 = tcc[0:P].rearrange("p (j c) -> p j c", j=2)
                        hnew3 = hnew[0:P].rearrange("p (j c) -> p j c", j=2)
                        nc.gpsimd.tensor_mul(hnew3, sig3[:, :, 512:768], tcc3)
                    else:
                        # split cell: critical half first (minimum latency),
                        # deferred half after the feed
                        sigc = cell_pool.tile([128, 768], F32, tag="sigc")
                        nc.scalar.activation(sigc[0:P], gc[0:P, 0:768],
                                             AF.Sigmoid)
                        tgc = cell_pool.tile([128, 256], F32, tag="tgc")
                        nc.scalar.activation(tgc[0:P], gc[0:P, 768:1024],
                                             AF.Tanh)
                        prodc = cell_pool.tile([128, 256], F32, tag="prodc")
                        nc.vector.tensor_mul(prodc[0:P], sigc[0:P, 0:256],
                                             tgc[0:P])
                        fcc = cell_pool.tile([128, 256], F32, tag="fcc")
                        nc.vector.tensor_mul(fcc[0:P], sigc[0:P, 256:512],
                                             cin_ap[:, 0:256])
                        nc.vector.tensor_add(cnew[0:P, 0:256], fcc[0:P],
                                             prodc[0:P])
                        tccc = cell_pool.tile([128, 256], F32, tag="tccc")
                        nc.scalar.activation(tccc[0:P], cnew[0:P, 0:256],
                                             AF.Tanh)
                        nc.vector.tensor_mul(hnew[0:P, 0:256],
                                             sigc[0:P, 512:768], tccc[0:P])
                        if L > 3:
                            feed_parent(stor[L - 1], gc, hnew[0:P, 0:256],
                                        cnew[0:P], P, pk)
                        # deferred half (fills engine gaps; GPSIMD-heavy)
                        sigd = cell_pool.tile([128, 768], F32, tag="sigd")
                        nc.scalar.activation(sigd[0:P], gd[0:P, 0:768],
                                             AF.Sigmoid)
                        tgd = cell_pool.tile([128, 256], F32, tag="tgd")
                        nc.scalar.activation(tgd[0:P], gd[0:P, 768:1024],
                                             AF.Tanh)
                        prodd = cell_pool.tile([128, 256], F32, tag="prodd")
                        nc.gpsimd.tensor_mul(prodd[0:P], sigd[0:P, 0:256],
                                             tgd[0:P])
                        fcd = cell_pool.tile([128, 256], F32, tag="fcd")
                        nc.gpsimd.tensor_mul(fcd[0:P], sigd[0:P, 256:512],
                                             cin_ap[:, 256:512])
                        nc.gpsimd.tensor_add(cnew[0:P, 256:512], fcd[0:P],
                                             prodd[0:P])
                        tccd = cell_pool.tile([128, 256], F32, tag="tccd")
                        nc.scalar.activation(tccd[0:P], cnew[0:P, 256:512],
                                             AF.Tanh)
                        nc.gpsimd.tensor_mul(hnew[0:P, 256:512],
                                             sigd[0:P, 512:768], tccd[0:P])

                    nc.sync.dma_start(
                        out_d[row_off + c0:row_off + c0 + P, :], hnew[0:P])

                    if L == 3:
                        nc.sync.dma_start(out_d[2047:2048, :], cnew[0:1])
                    elif fused:
                        feeds.append((gd, hnew, cnew, P, pk))

                for (gd, hnew, cnew, P, pk) in feeds:
                    feed_parent(stor[L - 1], gd, hnew[0:P, 0:256],
                                cnew[0:P], P, pk)

    nc.compile()
    return [k for k in din]


def _get_built():
    global _BUILT
    if _BUILT is None:
        nc = bacc.Bacc("TRN2", target_bir_lowering=False, debug=False,
                       num_devices=N_CORES)
        names = _build_program(nc)
        _BUILT = (nc, names)
    return _BUILT


def kernel(types, a_idx, b_idx, emb, W_ih, W_hh, b_ih, b_hh):
    types = np.asarray(types, np.int32)
    emb = np.asarray(emb, np.float32)
    W_ih = np.asarray(W_ih, np.float32)
    W_hh = np.asarray(W_hh, np.float32)
    b = np.asarray(b_ih, np.float32) + np.asarray(b_hh, np.float32)

    # ---- host weight reparameterization (O(V), no O(N) arithmetic) ----
    XT = (W_ih @ emb.T + b[:, None]).astype(np.float32)          # [2048, 32]
    c_leaf = _sigmoid(XT[0:512]) * np.tanh(XT[1024:1536])        # [512, 32]
    h_leaf = _sigmoid(XT[1536:2048]) * np.tanh(c_leaf)           # [512, 32]
    M_A = W_hh[:, 0:256] @ h_leaf[0:256]                         # [2048, 32]
    M_B = W_hh[:, 256:512] @ h_leaf[0:256]
    w13 = np.ascontiguousarray(
        np.vstack([M_A.T, M_B.T, XT.T])[:, GATE_PERM], np.float16)
    cl256 = np.ascontiguousarray(c_leaf[0:256].T)  # [32, 256]
    W_augT = np.vstack([W_hh.T, XT.T])[:, GATE_PERM]             # [544, 2048]
    wk = [np.ascontiguousarray(W_augT[i * 128:(i + 1) * 128], np.float16)
          for i in range(4)]
    woh = np.ascontiguousarray(W_augT[512:544], np.float16)
    eye = np.eye(128, dtype=np.float32)

    in_maps = []
    for j in range(N_CORES):
        # level 13: one-hots of (left-leaf, right-leaf, self) types
        base13 = (1 << 13) - 1 + j * 1024
        n = np.arange(base13, base13 + 1024)
        oh3 = np.zeros((96, 1024), np.float16)
        m = np.arange(1024)
        oh3[types[2 * n + 1], m] = 1.0
        oh3[32 + types[2 * n + 2], m] = 1.0
        oh3[64 + types[n], m] = 1.0
        cin13 = np.concatenate(
            [cl256[types[2 * n + 1]], cl256[types[2 * n + 2]]], axis=1)
        ohs = np.zeros((32, 1023), np.float16)
        for L in range(12, 2, -1):
            mm = 1 << (L - 3)
            basel = (1 << L) - 1 + j * mm
            off = OHS_OFF[L]
            ohs[types[basel:basel + mm], off + np.arange(mm)] = 1.0
        in_maps.append({
            "wk0": wk[0], "wk1": wk[1], "wk2": wk[2], "wk3": wk[3],
            "woh": woh, "w13": w13, "cin13": cin13,
            "oh3": oh3, "ohs": ohs, "eye": eye,
        })

    nc, _ = _get_built()
    res = run_bass_kernel_spmd(nc, in_maps, core_ids=list(range(N_CORES)))
    global LAST_RESULTS
    LAST_RESULTS = res

    out = np.empty((N, H2), np.float32)
    for j in range(N_CORES):
        r = res.results[j]["out"]
        off = 0
        for L in range(13, 2, -1):
            mm = 1 << (L - 3)
            basel = (1 << L) - 1 + j * mm
            out[basel:basel + mm] = r[off:off + mm]
            off += mm
    out[LEAF0:] = h_leaf.T[types[LEAF0:]]

    # top 7 nodes (levels 2..0) on host, exactly mirroring the reference
    Hs = np.zeros((15, H2), np.float32)
    Cs = np.zeros((15, H2), np.float32)
    for j in range(N_CORES):
        Hs[7 + j] = res.results[j]["out"][2046]
        Cs[7 + j] = res.results[j]["out"][2047]
    for n in range(6, -1, -1):
        a, bb = 2 * n + 1, 2 * n + 2
        hin = np.concatenate([Hs[a, :H], Hs[bb, :H]])
        cin = np.concatenate([Cs[a, :H], Cs[bb, :H]])
        gates = XT[:, types[n]] + W_hh @ hin
        ig, fg, gg, og = np.split(gates, 4)
        c_new = _sigmoid(fg) * cin + _sigmoid(ig) * np.tanh(gg)
        h_new = _sigmoid(og) * np.tanh(c_new)
        Hs[n] = h_new
        Cs[n] = c_new
        out[n] = h_new
    return out

